# revision 41
# baseline (speedup 1.0000x reference)
"""Trainium2 Bass kernel for nn_Attention (dense transformer spatial attention).

Reference computation (per batch b of 4):
  X = x[b] reshaped [256, 4096]                      (4096 = 64*64 pixels)
  QKV = w_qkv @ X -> [384, 4096]; q,k,v = split(QKV) each [128, 4096]
  per head h (4 heads x 32 dims): sim = (q_h*scale)^T k_h   [4096, 4096]
  attn = softmax(sim, axis=-1); out_h = attn @ v_h^T        [4096, 32]
  H = concat_heads -> [128, 4096]; out = w_out @ H + b_out  [256, 4096]

Sharding: 8 cores = (batch b in 0..3) x (query half qh in 0..1).
Each core gets full X_b (for K/V) plus its query-half slice, computes
attention output for its 2048 queries over all 4096 keys, and the final
projection. Gather on host is pure concatenation + transpose (device emits
[i, o] layout).

Device algorithm (per core), designed around engine rooflines (the kernel
is ScalarE-bound: 33.5M softmax exp evaluations per core at 1 elem/lane/
cycle is the hard floor, ~240us; PE/DVE/DMA work hides underneath):
  - Matmuls in float32r (FP22 multiply, 1-pass full-rate on the PE); the
    AV stage uses bf16 operands (the fused f32r weight-load path cannot
    target col-offset PSUM, and mixed f32r/bf16 operands are rejected).
  - sim is computed TRANSPOSED: simT[j, i] = sum_d k[d,j] q[d,i], via
    4-way row-packed K=32 matmuls (one per head, tile_position=(32h,0)),
    so no transposes of the big attention matrix are ever needed.
  - softmax: max-subtraction is skipped (|scale*sim| <~ 20 always, exp is
    safe in f32); scale is folded into the ScalarE exp activation.
  - denominator: V^T is augmented with a ones column then zero-padded to
    M=64, so the AV matmul computes sum_j exp*v AND sum_j exp in one pass.
  - AV: out^T[d_aug, i] = sum_j vTaug[j, d_aug] expT[j, i], accumulated
    over j tiles in PSUM; heads col-packed in pairs at tile_position
    (0,0)/(0,64). Every matmul accumulation group owns whole PSUM banks
    (has_written zeroing is 2KB-region granular, NOT per element).
  - normalization + projection emit outT [i, o] so the per-query softmax
    denominator is applied with plain DVE ops; host transposes back.
  - PSUM budget (8 banks): sim quad [128,2048] = 4 banks (single-buffered;
    the sim->exp WAR serialization ~380ns/step is the price), AV pair
    accumulators 2x[128,1024] = 4 banks.
"""

import numpy as np

import concourse.bacc as bacc
import concourse.bass as bass
import concourse.mybir as mybir
import concourse.tile as tile
from concourse.bass_utils import run_bass_kernel_spmd


F32 = mybir.dt.float32
F32R = mybir.dt.float32r
BF16 = mybir.dt.bfloat16

HEADS = 4
DH = 32                      # dim per head
C = 256                      # input channels
NJ = 4096                    # keys per batch (64*64)
NI = 2048                    # queries per core (half of 4096)
JT = 128                     # j tile (partition dim of simT)
NJT = NJ // JT               # 32 j tiles
NT = 512                     # i tile for sim/exp/AV matmuls
CHUNK = 1024                 # i chunk held in AV psum accumulators
NCHUNK = NI // CHUNK         # 2
SCALE = float(DH) ** -0.5
BW = NJ + NI + 3 * 128       # blob256 width


def build_kernel(dbg=False):
    nc = bacc.Bacc("TRN2", debug=False, num_devices=8)

    # blob256 columns: [wqkvT (384) | xq (2048) | x (4096)] - weights and
    # query slice first so the q projection can start while x still streams
    # blob128 columns: [woutA (256) | woutB (256) | bias replicated (256)]
    blob256_d = nc.dram_tensor("blob256", [C, BW], F32R, kind="ExternalInput").ap()
    blob128_d = nc.dram_tensor("blob128", [128, 3 * C], F32R, kind="ExternalInput").ap()
    out_d = nc.dram_tensor("out_t", [NI, C], F32, kind="ExternalOutput").ap()
    # DRAM bounce buffer for partition-broadcasting softmax reciprocals
    # (SBUF->SBUF DMA cannot have a zero partition step on the source).
    rscr_d = nc.dram_tensor("rbscratch", [NCHUNK, 4, CHUNK], F32).ap()
    if dbg:
        dumps = {n: nc.dram_tensor("dump_" + n, s, d, kind="ExternalOutput").ap()
                 for n, s, d in [
                     ("q", [128, NI], F32), ("k", [128, NJ], F32),
                     ("vT", [128, NJT * HEADS * 64], BF16),
                     ("h1", [128, NI], F32), ("h2", [128, NI], F32),
                     ("rb1", [128, CHUNK], F32), ("rb2", [128, CHUNK], F32)]}

    with tile.TileContext(nc) as tc:
        with (
            tc.tile_pool(name="singles", bufs=1) as singles,
            tc.tile_pool(name="expp", bufs=3) as expp,
            tc.tile_pool(name="exp6", bufs=8) as exp6,
            tc.tile_pool(name="outp", bufs=3) as outp,
            tc.tile_pool(name="psim", bufs=1, space="PSUM") as psim,
            tc.tile_pool(name="pav", bufs=2, space="PSUM") as pav,
        ):
            # ---- resident SBUF tensors ----
            blob_sb = singles.tile([128, 2, BW], F32R)    # w|xq|x, 2 c-tiles
            w_sb = blob_sb[:, :, 0:3 * 128]
            xq_sb = blob_sb[:, :, 3 * 128:3 * 128 + NI]
            x_sb = blob_sb[:, :, 3 * 128 + NI:BW]
            b128_sb = singles.tile([128, 3 * C], F32R)
            woutA_sb = b128_sb[:, 0:C]
            woutB_sb = b128_sb[:, C:2 * C]
            bias_sb = b128_sb[:, 2 * C:3 * C]
            q_sb = singles.tile([128, NI], F32R)          # q rows = 4h x 32d
            k_sb = singles.tile([128, NJ], F32R)
            # vT padded to 64 cols: [v dims (32) | ones (1) | zeros (31)]
            # (M=64 keeps the (0,64) col-tiled AV matmul ISA-valid; matmul
            # cost is N-bound so the padding is free)
            # bf16: the fused f32r weight-load path cannot target col-offset
            # PSUM (tile_position (0,64)); bf16 uses the normal LDWEIGHTS path
            vT_sb = singles.tile([128, NJT, HEADS, 64], BF16)
            h1_sb = singles.tile([128, NI], F32R)         # heads 0/1 at rows 0-31/64-95
            h2_sb = singles.tile([128, NI], F32R)         # heads 2/3 at rows 0-31/64-95
            rb1_sb = singles.tile([128, CHUNK], F32)     # 1/denom bcast for h1 rows
            rb2_sb = singles.tile([128, CHUNK], F32)

            # single SWDGE queue -> one semaphore for all initial loads
            # (HWDGE round-robins queues and early matmuls then exceed the
            # per-instruction sync-wait slot limit)
            # One contiguous DMA per c-tile: every matmul then transitively
            # depends on exactly ONE DMA (walrus allows only ~2 semaphore
            # waits per instruction, so the wait sets must stay tiny).
            W0 = 3 * 128 + NI          # w + xq prefix
            for ct in range(2):
                nc.sync.dma_start(out=blob_sb[:, ct, 0:W0],
                                  in_=blob256_d[ct * 128:(ct + 1) * 128, 0:W0])
            for ct in range(2):
                for xh in range(2):
                    lo = W0 + xh * (NJ // 2)
                    nc.sync.dma_start(out=blob_sb[:, ct, lo:lo + NJ // 2],
                                      in_=blob256_d[ct * 128:(ct + 1) * 128,
                                                    lo:lo + NJ // 2])
            nc.sync.dma_start(out=b128_sb, in_=blob128_d)

            nc.vector.memset(h1_sb[:, :].bitcast(F32), 0.0)  # unused rows stay 0
            nc.vector.memset(h2_sb[:, :].bitcast(F32), 0.0)
            nc.vector.memset(vT_sb, 0.0)                # zero padding
            nc.vector.memset(vT_sb[:, :, :, DH], 1.0)   # ones column
            nc.vector.memset(rb1_sb, 0.0)
            nc.vector.memset(rb2_sb, 0.0)

            # trigger the ScalarE exp table load (~2.7us) during phase 1
            # instead of at the first real softmax activation
            warm = singles.tile([1, 1], F32)
            nc.vector.memset(warm, 0.0)
            nc.scalar.activation(warm, warm, mybir.ActivationFunctionType.Exp)

            # ---- phase 1: qkv projections ----
            # ordered to match DMA arrival: q needs only w+xq (first DMAs),
            # k-half0/vT(0-15) need x-half0, the rest needs x-half1
            psq = psim.tile([128, NI], F32, tag="sim")
            for nt in range(NI // 512):
                for ct in range(2):
                    nc.tensor.matmul(
                        psq[:, nt * 512:(nt + 1) * 512],
                        lhsT=w_sb[:, ct, 0:128],
                        rhs=xq_sb[:, ct, nt * 512:(nt + 1) * 512],
                        start=(ct == 0), stop=(ct == 1),
                    )
            nc.vector.tensor_copy(q_sb, psq)

            def emit_k_half(half):
                psk = psim.tile([128, NI], F32, tag="sim")
                for nt in range(4):
                    col = half * NI + nt * 512
                    for ct in range(2):
                        nc.tensor.matmul(
                            psk[:, nt * 512:(nt + 1) * 512],
                            lhsT=w_sb[:, ct, 128:256],
                            rhs=x_sb[:, ct, col:col + 512],
                            start=(ct == 0), stop=(ct == 1),
                        )
                nc.vector.tensor_copy(k_sb[:, half * NI:(half + 1) * NI], psk)

            def emit_vt_range(lo, hi):
                # vT[j, vc] = sum_c x[c, j] wv[vc, c], one [128, 128] tile per jt
                for jt in range(lo, hi):
                    psv = pav.tile([128, 128], F32, tag="av")
                    for ct in range(2):
                        nc.tensor.matmul(
                            psv,
                            lhsT=x_sb[:, ct, jt * JT:(jt + 1) * JT],
                            rhs=w_sb[:, ct, 256:384],
                            start=(ct == 0), stop=(ct == 1),
                        )
                    nc.vector.tensor_copy(vT_sb[:, jt, :, 0:DH], psv)

            emit_k_half(0)
            emit_vt_range(0, NJT // 2)
            emit_k_half(1)
            emit_vt_range(NJT // 2, NJT)

            # ---- phase 2: attention main loop ----
            for chunk in range(NCHUNK):
                co = chunk * CHUNK
                avA = pav.tile([128, CHUNK], F32, tag="av")  # heads 0 @0-32, 1 @64-96
                avB = pav.tile([128, CHUNK], F32, tag="av")  # heads 2 @0-32, 3 @64-96
                def emit_av(ex, jt, nt):
                    for h in range(HEADS):
                        av = avA if h < 2 else avB
                        po = 64 * (h % 2)
                        nc.tensor.matmul(
                            av[po:po + 64, nt * NT:(nt + 1) * NT],
                            lhsT=vT_sb[:, jt, h, :],
                            rhs=ex[:, h * NT:(h + 1) * NT],
                            start=(jt == 0), stop=(jt == NJT - 1),
                            tile_position=(0, po),
                            skip_group_check=True,
                        )

                # AV for step s is emitted after sim for step s+1 so the PE
                # unblocks the ScalarE exp (the critical path) first
                pending = None
                for jt in range(NJT):
                    for nt in range(CHUNK // NT):
                        io = co + nt * NT
                        sim = psim.tile([128, HEADS * NT], F32, tag="sim")
                        for h in range(HEADS):
                            nc.tensor.matmul(
                                sim[:, h * NT:(h + 1) * NT],
                                lhsT=k_sb[h * DH:(h + 1) * DH, jt * JT:(jt + 1) * JT],
                                rhs=q_sb[h * DH:(h + 1) * DH, io:io + NT],
                                start=True, stop=True,
                                tile_position=(h * DH, 0),
                            )
                        ex = exp6.tile([128, HEADS * NT], BF16, tag="exp")
                        nc.scalar.activation(ex, sim, mybir.ActivationFunctionType.Exp,
                                             scale=SCALE)
                        if pending is not None:
                            emit_av(*pending)
                        pending = (ex, jt, nt)
                emit_av(*pending)

                # softmax denominators (rows 32 & 96 of the av tiles):
                # bounce the 4 rows through DRAM to pack them into [4, CHUNK]
                # (reciprocal is free-dim bound: one packed call is 4x
                # cheaper than four [1, CHUNK] calls), then reciprocal,
                # bounce back, and partition-broadcast over each head's rows.
                den4 = expp.tile([4, CHUNK], F32, tag="rc")
                rc4 = expp.tile([4, CHUNK], F32, tag="rc")
                dstg = expp.tile([128, 2 * CHUNK], F32, tag="dstg")
                for idx, av in enumerate((avA, avB)):
                    for pi, po in enumerate((0, 64)):
                        h4 = idx * 2 + pi
                        cs = idx * CHUNK
                        # DMA cannot read PSUM: stage the row via DVE
                        # (same partition, pair-tiles split by free offset)
                        nc.vector.tensor_copy(dstg[po + DH:po + DH + 1, cs:cs + CHUNK],
                                              av[po + DH:po + DH + 1, :])
                        nc.sync.dma_start(out=rscr_d[chunk, h4, :],
                                          in_=dstg[po + DH:po + DH + 1, cs:cs + CHUNK])
                nc.sync.dma_start(out=den4, in_=rscr_d[chunk, :, :])
                nc.vector.reciprocal(out=rc4, in_=den4)
                nc.sync.dma_start(out=rscr_d[chunk, :, :], in_=rc4)
                for idx, rb in enumerate((rb1_sb, rb2_sb)):
                    for pi, po in enumerate((0, 64)):
                        h4 = idx * 2 + pi
                        nc.sync.dma_start(
                            out=rb[po:po + DH, :],
                            in_=rscr_d[chunk, h4:h4 + 1, :].to_broadcast((DH, CHUNK)),
                        )
                # fused normalize + PSUM->SBUF copy
                for (av, rb, hsb) in ((avA, rb1_sb, h1_sb), (avB, rb2_sb, h2_sb)):
                    for po in (0, 64):
                        nc.vector.tensor_tensor(
                            out=hsb[po:po + DH, co:co + CHUNK],
                            in0=av[po:po + DH, :],
                            in1=rb[po:po + DH, :],
                            op=mybir.AluOpType.mult,
                        )

                # ---- output projection for this chunk ----
                for it in range(CHUNK // 128):
                    io = co + it * 128
                    pj = pav.tile([128, C], F32, tag="av")
                    nc.tensor.matmul(pj, lhsT=h1_sb[:, io:io + 128],
                                     rhs=woutA_sb, start=True, stop=False)
                    nc.tensor.matmul(pj, lhsT=h2_sb[:, io:io + 128],
                                     rhs=woutB_sb, start=False, stop=True)
                    ot = outp.tile([128, C], F32, tag="out")
                    nc.vector.tensor_tensor(out=ot, in0=pj, in1=bias_sb,
                                            op=mybir.AluOpType.add)
                    nc.sync.dma_start(out=out_d[io:io + 128, :], in_=ot)

            if dbg:
                nc.sync.dma_start(out=dumps["q"], in_=q_sb[:, :].bitcast(F32))
                nc.sync.dma_start(out=dumps["k"], in_=k_sb[:, :].bitcast(F32))
                nc.sync.dma_start(out=dumps["vT"],
                                  in_=vT_sb[:, :, :, :].rearrange("p a b c -> p (a b c)"))
                nc.sync.dma_start(out=dumps["h1"], in_=h1_sb[:, :].bitcast(F32))
                nc.sync.dma_start(out=dumps["h2"], in_=h2_sb[:, :].bitcast(F32))
                nc.sync.dma_start(out=dumps["rb1"], in_=rb1_sb)
                nc.sync.dma_start(out=dumps["rb2"], in_=rb2_sb)

    nc.compile()
    return nc


_NC = None


def _get_nc():
    global _NC
    if _NC is None:
        _NC = build_kernel()
    return _NC


def make_in_maps(x, w_qkv, w_out, b_out):
    x = np.ascontiguousarray(np.asarray(x, dtype=np.float32))
    w_qkv = np.asarray(w_qkv, dtype=np.float32)
    w_out = np.asarray(w_out, dtype=np.float32)
    b_out = np.asarray(b_out, dtype=np.float32)

    wqkvT = w_qkv.T                                       # [256, 384]
    woutT = w_out.T                                       # [128 hidden, 256]
    # projection weights permuted to the AV psum partition layout:
    # A: rows 0-31 = head0, rows 64-95 = head1; B: head2, head3; rest zero
    woutA = np.zeros((128, C), np.float32)
    woutB = np.zeros((128, C), np.float32)
    woutA[0:32] = woutT[0:32]
    woutA[64:96] = woutT[32:64]
    woutB[0:32] = woutT[64:96]
    woutB[64:96] = woutT[96:128]
    blob128 = np.ascontiguousarray(
        np.concatenate([woutA, woutB,
                        np.broadcast_to(b_out[None, :], (128, C))], axis=1))

    in_maps = []
    for core in range(8):
        b, qh = divmod(core, 2)
        xb = x[b].reshape(C, NJ)
        xqb = xb[:, qh * NI:(qh + 1) * NI]
        blob256 = np.ascontiguousarray(
            np.concatenate([wqkvT, xqb, xb], axis=1))
        in_maps.append({"blob256": blob256, "blob128": blob128})
    return in_maps


def run_spmd(x, w_qkv, w_out, b_out, **kw):
    nc = _get_nc()
    in_maps = make_in_maps(x, w_qkv, w_out, b_out)
    return run_bass_kernel_spmd(nc, in_maps, core_ids=list(range(8)), **kw)


def assemble(results):
    out = np.empty((4, C, NJ), np.float32)
    for core in range(8):
        b, qh = divmod(core, 2)
        out[b, :, qh * NI:(qh + 1) * NI] = results[core]["out_t"].T
    return out.reshape(4, C, 64, 64)


def kernel(x, w_qkv, w_out, b_out):
    res = run_spmd(x, w_qkv, w_out, b_out)
    return assemble(res.results)


# revision 42
# speedup vs baseline: 1.3808x; 1.3808x over previous
"""Trainium2 Bass kernel for nn_Attention (dense transformer spatial attention).

Reference computation (per batch b of 4):
  X = x[b] reshaped [256, 4096]                      (4096 = 64*64 pixels)
  QKV = w_qkv @ X -> [384, 4096]; q,k,v = split(QKV) each [128, 4096]
  per head h (4 heads x 32 dims): sim = (q_h*scale)^T k_h   [4096, 4096]
  attn = softmax(sim, axis=-1); out_h = attn @ v_h^T        [4096, 32]
  H = concat_heads -> [128, 4096]; out = w_out @ H + b_out  [256, 4096]

Sharding: 8 cores = (batch b in 0..3) x (query half qh in 0..1).
Each core gets full X_b (for K/V) plus its query-half slice, computes
attention output for its 2048 queries over all 4096 keys, and the final
projection. Gather on host is pure concatenation + transpose (device emits
[i, o] layout).

Device algorithm (per core), designed around engine rooflines (the kernel
is ScalarE-bound: 33.5M softmax exp evaluations per core at 1 elem/lane/
cycle is the hard floor, ~240us; PE/DVE/DMA work hides underneath):
  - Matmuls in float32r (FP22 multiply, 1-pass full-rate on the PE); the
    AV stage uses bf16 operands (the fused f32r weight-load path cannot
    target col-offset PSUM, and mixed f32r/bf16 operands are rejected).
  - sim is computed TRANSPOSED: simT[j, i] = sum_d k[d,j] q[d,i], via
    4-way row-packed K=32 matmuls (one per head, tile_position=(32h,0)),
    so no transposes of the big attention matrix are ever needed.
  - softmax: max-subtraction is skipped (|scale*sim| <~ 20 always, exp is
    safe in f32); scale is folded into the ScalarE exp activation.
  - denominator: V^T is augmented with a ones column then zero-padded to
    M=64, so the AV matmul computes sum_j exp*v AND sum_j exp in one pass.
  - AV: out^T[d_aug, i] = sum_j vTaug[j, d_aug] expT[j, i], accumulated
    over j tiles in PSUM; heads col-packed in pairs at tile_position
    (0,0)/(0,64). Every matmul accumulation group owns whole PSUM banks
    (has_written zeroing is 2KB-region granular, NOT per element).
  - normalization + projection emit outT [i, o] so the per-query softmax
    denominator is applied with plain DVE ops; host transposes back.
  - PSUM budget (8 banks): sim quad [128,2048] = 4 banks (single-buffered;
    the sim->exp WAR serialization ~380ns/step is the price), AV pair
    accumulators 2x[128,1024] = 4 banks.
"""

import numpy as np

import concourse.bacc as bacc
import concourse.bass as bass
import concourse.mybir as mybir
import concourse.tile as tile
from concourse.bass_utils import run_bass_kernel_spmd


F32 = mybir.dt.float32
F32R = mybir.dt.float32r
BF16 = mybir.dt.bfloat16

HEADS = 4
DH = 32                      # dim per head
C = 256                      # input channels
NJ = 4096                    # keys per batch (64*64)
NI = 2048                    # queries per core (half of 4096)
JT = 128                     # j tile (partition dim of simT)
NJT = NJ // JT               # 32 j tiles
NT = 512                     # i tile for sim/exp/AV matmuls
CHUNK = 1024                 # i chunk held in AV psum accumulators
NCHUNK = NI // CHUNK         # 2
SCALE = float(DH) ** -0.5
BW = NJ + NI + 3 * 128       # blob256 width


def build_kernel(dbg=False):
    nc = bacc.Bacc("TRN2", debug=False, num_devices=8)

    # blob256 columns: [wqkvT (384) | xq (2048) | x (4096)] - weights and
    # query slice first so the q projection can start while x still streams
    # blob128 columns: [woutA (256) | woutB (256) | bias replicated (256)]
    blob256_d = nc.dram_tensor("blob256", [C, BW], F32R, kind="ExternalInput").ap()
    blob128_d = nc.dram_tensor("blob128", [128, 3 * C], F32R, kind="ExternalInput").ap()
    out_d = nc.dram_tensor("out_t", [NI, C], F32, kind="ExternalOutput").ap()
    # DRAM bounce buffer for partition-broadcasting softmax reciprocals
    # (SBUF->SBUF DMA cannot have a zero partition step on the source).
    rscr_d = nc.dram_tensor("rbscratch", [NCHUNK, 4, CHUNK], F32).ap()
    if dbg:
        dumps = {n: nc.dram_tensor("dump_" + n, s, d, kind="ExternalOutput").ap()
                 for n, s, d in [
                     ("q", [128, NI], F32), ("k", [128, NJ], F32),
                     ("vT", [128, NJT * HEADS * 64], BF16),
                     ("h1", [128, NI], F32), ("h2", [128, NI], F32),
                     ("rb1", [128, CHUNK], F32), ("rb2", [128, CHUNK], F32)]}

    with tile.TileContext(nc) as tc:
        with (
            tc.tile_pool(name="singles", bufs=1) as singles,
            tc.tile_pool(name="expp", bufs=3) as expp,
            tc.tile_pool(name="exp6", bufs=8) as exp6,
            tc.tile_pool(name="outp", bufs=3) as outp,
            tc.tile_pool(name="psim", bufs=1, space="PSUM") as psim,
            tc.tile_pool(name="pav", bufs=2, space="PSUM") as pav,
        ):
            # ---- resident SBUF tensors ----
            blob_sb = singles.tile([128, 2, BW], F32R)    # w|xq|x, 2 c-tiles
            w_sb = blob_sb[:, :, 0:3 * 128]
            xq_sb = blob_sb[:, :, 3 * 128:3 * 128 + NI]
            x_sb = blob_sb[:, :, 3 * 128 + NI:BW]
            b128_sb = singles.tile([128, 3 * C], F32R)
            woutA_sb = b128_sb[:, 0:C]
            woutB_sb = b128_sb[:, C:2 * C]
            bias_sb = b128_sb[:, 2 * C:3 * C]
            q_sb = singles.tile([128, NI], F32R)          # q rows = 4h x 32d
            k_sb = singles.tile([128, NJ], F32R)
            # vT padded to 64 cols: [v dims (32) | ones (1) | zeros (31)]
            # (M=64 keeps the (0,64) col-tiled AV matmul ISA-valid; matmul
            # cost is N-bound so the padding is free)
            # bf16: the fused f32r weight-load path cannot target col-offset
            # PSUM (tile_position (0,64)); bf16 uses the normal LDWEIGHTS path
            vT_sb = singles.tile([128, NJT, HEADS, 64], BF16)
            h1_sb = singles.tile([128, NI], F32R)         # heads 0/1 at rows 0-31/64-95
            h2_sb = singles.tile([128, NI], F32R)         # heads 2/3 at rows 0-31/64-95
            rb1_sb = singles.tile([128, CHUNK], F32)     # 1/denom bcast for h1 rows
            rb2_sb = singles.tile([128, CHUNK], F32)

            # single SWDGE queue -> one semaphore for all initial loads
            # (HWDGE round-robins queues and early matmuls then exceed the
            # per-instruction sync-wait slot limit)
            # One contiguous DMA per c-tile: every matmul then transitively
            # depends on exactly ONE DMA (walrus allows only ~2 semaphore
            # waits per instruction, so the wait sets must stay tiny).
            W0 = 3 * 128 + NI          # w + xq prefix
            for ct in range(2):
                nc.sync.dma_start(out=blob_sb[:, ct, 0:W0],
                                  in_=blob256_d[ct * 128:(ct + 1) * 128, 0:W0])
            for ct in range(2):
                for xh in range(2):
                    lo = W0 + xh * (NJ // 2)
                    nc.sync.dma_start(out=blob_sb[:, ct, lo:lo + NJ // 2],
                                      in_=blob256_d[ct * 128:(ct + 1) * 128,
                                                    lo:lo + NJ // 2])
            nc.sync.dma_start(out=b128_sb, in_=blob128_d)

            nc.vector.memset(h1_sb[:, :].bitcast(F32), 0.0)  # unused rows stay 0
            nc.vector.memset(h2_sb[:, :].bitcast(F32), 0.0)
            nc.vector.memset(vT_sb, 0.0)                # zero padding
            nc.vector.memset(vT_sb[:, :, :, DH], 1.0)   # ones column
            nc.vector.memset(rb1_sb, 0.0)
            nc.vector.memset(rb2_sb, 0.0)

            # trigger the ScalarE exp table load (~2.7us) during phase 1
            # instead of at the first real softmax activation
            warm = singles.tile([1, 1], F32)
            nc.vector.memset(warm, 0.0)
            nc.scalar.activation(warm, warm, mybir.ActivationFunctionType.Exp)

            # ---- phase 1: qkv projections ----
            # ordered to match DMA arrival: q needs only w+xq (first DMAs),
            # k-half0/vT(0-15) need x-half0, the rest needs x-half1
            for qg in range(2):
                psq = psim.tile([128, NI // 2], F32, tag="simA" if qg == 0 else "simB")
                for nt in range(2):
                    col = qg * 1024 + nt * 512
                    for ct in range(2):
                        nc.tensor.matmul(
                            psq[:, nt * 512:(nt + 1) * 512],
                            lhsT=w_sb[:, ct, 0:128],
                            rhs=xq_sb[:, ct, col:col + 512],
                            start=(ct == 0), stop=(ct == 1),
                        )
                nc.vector.tensor_copy(q_sb[:, qg * 1024:(qg + 1) * 1024], psq)

            def emit_k_half(half):
                for kg in range(2):
                    psk = psim.tile([128, NI // 2], F32,
                                    tag="simA" if kg == 0 else "simB")
                    for nt in range(2):
                        col = half * NI + kg * 1024 + nt * 512
                        for ct in range(2):
                            nc.tensor.matmul(
                                psk[:, nt * 512:(nt + 1) * 512],
                                lhsT=w_sb[:, ct, 128:256],
                                rhs=x_sb[:, ct, col:col + 512],
                                start=(ct == 0), stop=(ct == 1),
                            )
                    nc.vector.tensor_copy(
                        k_sb[:, half * NI + kg * 1024:half * NI + (kg + 1) * 1024],
                        psk)

            def emit_vt_range(lo, hi):
                # vT[j, vc] = sum_c x[c, j] wv[vc, c], one [128, 128] tile per jt
                for jt in range(lo, hi):
                    psv = pav.tile([128, 128], F32, tag="av")
                    for ct in range(2):
                        nc.tensor.matmul(
                            psv,
                            lhsT=x_sb[:, ct, jt * JT:(jt + 1) * JT],
                            rhs=w_sb[:, ct, 256:384],
                            start=(ct == 0), stop=(ct == 1),
                        )
                    nc.vector.tensor_copy(vT_sb[:, jt, :, 0:DH], psv)

            emit_k_half(0)
            emit_vt_range(0, NJT // 2)
            emit_k_half(1)
            emit_vt_range(NJT // 2, NJT)

            # ---- phase 2: attention main loop ----
            for chunk in range(NCHUNK):
                co = chunk * CHUNK
                avA = pav.tile([128, CHUNK], F32, tag="av")  # heads 0 @0-32, 1 @64-96
                avB = pav.tile([128, CHUNK], F32, tag="av")  # heads 2 @0-32, 3 @64-96
                def emit_av(ex, jt, nt):
                    for h in range(HEADS):
                        av = avA if h < 2 else avB
                        po = 64 * (h % 2)
                        nc.tensor.matmul(
                            av[po:po + 64, nt * NT:(nt + 1) * NT],
                            lhsT=vT_sb[:, jt, h, :],
                            rhs=ex[:, h * NT:(h + 1) * NT],
                            start=(jt == 0), stop=(jt == NJT - 1),
                            tile_position=(0, po),
                            skip_group_check=True,
                        )

                # AV for step s is emitted after sim for step s+1 so the PE
                # unblocks the ScalarE exp (the critical path) first
                # the quad/exp pair is split in two halves over SEPARATE
                # psum tiles (tags simA/simB): the next step's heads-0/1
                # matmuls overlap the current heads-2/3 exp, so the PE never
                # sits on the ScalarE critical path
                pending = None
                for jt in range(NJT):
                    for nt in range(CHUNK // NT):
                        io = co + nt * NT
                        ex = exp6.tile([128, HEADS * NT], BF16, tag="exp")
                        for grp, tag in ((0, "simA"), (1, "simB")):
                            sim = psim.tile([128, 2 * NT], F32, tag=tag)
                            for hi in range(2):
                                h = grp * 2 + hi
                                nc.tensor.matmul(
                                    sim[:, hi * NT:(hi + 1) * NT],
                                    lhsT=k_sb[h * DH:(h + 1) * DH,
                                              jt * JT:(jt + 1) * JT],
                                    rhs=q_sb[h * DH:(h + 1) * DH, io:io + NT],
                                    start=True, stop=True,
                                    tile_position=(h * DH, 0),
                                )
                            nc.scalar.activation(
                                ex[:, grp * 2 * NT:(grp + 1) * 2 * NT], sim,
                                mybir.ActivationFunctionType.Exp, scale=SCALE)
                        if pending is not None:
                            emit_av(*pending)
                        pending = (ex, jt, nt)
                emit_av(*pending)

                # softmax denominators (rows 32 & 96 of the av tiles):
                # bounce the 4 rows through DRAM to pack them into [4, CHUNK]
                # (reciprocal is free-dim bound: one packed call is 4x
                # cheaper than four [1, CHUNK] calls), then reciprocal,
                # bounce back, and partition-broadcast over each head's rows.
                den4 = expp.tile([4, CHUNK], F32, tag="rc")
                rc4 = expp.tile([4, CHUNK], F32, tag="rc")
                dstg = expp.tile([128, 2 * CHUNK], F32, tag="dstg")
                for idx, av in enumerate((avA, avB)):
                    for pi, po in enumerate((0, 64)):
                        h4 = idx * 2 + pi
                        cs = idx * CHUNK
                        # DMA cannot read PSUM: stage the row via DVE
                        # (same partition, pair-tiles split by free offset)
                        nc.vector.tensor_copy(dstg[po + DH:po + DH + 1, cs:cs + CHUNK],
                                              av[po + DH:po + DH + 1, :])
                        nc.sync.dma_start(out=rscr_d[chunk, h4, :],
                                          in_=dstg[po + DH:po + DH + 1, cs:cs + CHUNK])
                nc.sync.dma_start(out=den4, in_=rscr_d[chunk, :, :])
                nc.vector.reciprocal(out=rc4, in_=den4)
                nc.sync.dma_start(out=rscr_d[chunk, :, :], in_=rc4)
                for idx, rb in enumerate((rb1_sb, rb2_sb)):
                    for pi, po in enumerate((0, 64)):
                        h4 = idx * 2 + pi
                        nc.sync.dma_start(
                            out=rb[po:po + DH, :],
                            in_=rscr_d[chunk, h4:h4 + 1, :].to_broadcast((DH, CHUNK)),
                        )
                # fused normalize + PSUM->SBUF copy
                for (av, rb, hsb) in ((avA, rb1_sb, h1_sb), (avB, rb2_sb, h2_sb)):
                    for po in (0, 64):
                        nc.vector.tensor_tensor(
                            out=hsb[po:po + DH, co:co + CHUNK],
                            in0=av[po:po + DH, :],
                            in1=rb[po:po + DH, :],
                            op=mybir.AluOpType.mult,
                        )

                # ---- output projection for this chunk ----
                for it in range(CHUNK // 128):
                    io = co + it * 128
                    pj = pav.tile([128, C], F32, tag="av")
                    nc.tensor.matmul(pj, lhsT=h1_sb[:, io:io + 128],
                                     rhs=woutA_sb, start=True, stop=False)
                    nc.tensor.matmul(pj, lhsT=h2_sb[:, io:io + 128],
                                     rhs=woutB_sb, start=False, stop=True)
                    ot = outp.tile([128, C], F32, tag="out")
                    nc.vector.tensor_tensor(out=ot, in0=pj, in1=bias_sb,
                                            op=mybir.AluOpType.add)
                    nc.sync.dma_start(out=out_d[io:io + 128, :], in_=ot)

            if dbg:
                nc.sync.dma_start(out=dumps["q"], in_=q_sb[:, :].bitcast(F32))
                nc.sync.dma_start(out=dumps["k"], in_=k_sb[:, :].bitcast(F32))
                nc.sync.dma_start(out=dumps["vT"],
                                  in_=vT_sb[:, :, :, :].rearrange("p a b c -> p (a b c)"))
                nc.sync.dma_start(out=dumps["h1"], in_=h1_sb[:, :].bitcast(F32))
                nc.sync.dma_start(out=dumps["h2"], in_=h2_sb[:, :].bitcast(F32))
                nc.sync.dma_start(out=dumps["rb1"], in_=rb1_sb)
                nc.sync.dma_start(out=dumps["rb2"], in_=rb2_sb)

    nc.compile()
    return nc


_NC = None


def _get_nc():
    global _NC
    if _NC is None:
        _NC = build_kernel()
    return _NC


def make_in_maps(x, w_qkv, w_out, b_out):
    x = np.ascontiguousarray(np.asarray(x, dtype=np.float32))
    w_qkv = np.asarray(w_qkv, dtype=np.float32)
    w_out = np.asarray(w_out, dtype=np.float32)
    b_out = np.asarray(b_out, dtype=np.float32)

    wqkvT = w_qkv.T                                       # [256, 384]
    woutT = w_out.T                                       # [128 hidden, 256]
    # projection weights permuted to the AV psum partition layout:
    # A: rows 0-31 = head0, rows 64-95 = head1; B: head2, head3; rest zero
    woutA = np.zeros((128, C), np.float32)
    woutB = np.zeros((128, C), np.float32)
    woutA[0:32] = woutT[0:32]
    woutA[64:96] = woutT[32:64]
    woutB[0:32] = woutT[64:96]
    woutB[64:96] = woutT[96:128]
    blob128 = np.ascontiguousarray(
        np.concatenate([woutA, woutB,
                        np.broadcast_to(b_out[None, :], (128, C))], axis=1))

    in_maps = []
    for core in range(8):
        b, qh = divmod(core, 2)
        xb = x[b].reshape(C, NJ)
        xqb = xb[:, qh * NI:(qh + 1) * NI]
        blob256 = np.ascontiguousarray(
            np.concatenate([wqkvT, xqb, xb], axis=1))
        in_maps.append({"blob256": blob256, "blob128": blob128})
    return in_maps


def run_spmd(x, w_qkv, w_out, b_out, **kw):
    nc = _get_nc()
    in_maps = make_in_maps(x, w_qkv, w_out, b_out)
    return run_bass_kernel_spmd(nc, in_maps, core_ids=list(range(8)), **kw)


def assemble(results):
    out = np.empty((4, C, NJ), np.float32)
    for core in range(8):
        b, qh = divmod(core, 2)
        out[b, :, qh * NI:(qh + 1) * NI] = results[core]["out_t"].T
    return out.reshape(4, C, 64, 64)


def kernel(x, w_qkv, w_out, b_out):
    res = run_spmd(x, w_qkv, w_out, b_out)
    return assemble(res.results)


# revision 43
# speedup vs baseline: 1.3843x; 1.0025x over previous
"""Trainium2 Bass kernel for nn_Attention (dense transformer spatial attention).

Reference computation (per batch b of 4):
  X = x[b] reshaped [256, 4096]                      (4096 = 64*64 pixels)
  QKV = w_qkv @ X -> [384, 4096]; q,k,v = split(QKV) each [128, 4096]
  per head h (4 heads x 32 dims): sim = (q_h*scale)^T k_h   [4096, 4096]
  attn = softmax(sim, axis=-1); out_h = attn @ v_h^T        [4096, 32]
  H = concat_heads -> [128, 4096]; out = w_out @ H + b_out  [256, 4096]

Sharding: 8 cores = (batch b in 0..3) x (query half qh in 0..1).
Each core gets full X_b (for K/V) plus its query-half slice, computes
attention output for its 2048 queries over all 4096 keys, and the final
projection. Gather on host is pure concatenation + transpose (device emits
[i, o] layout).

Device algorithm (per core), designed around engine rooflines (the kernel
is ScalarE-bound: 33.5M softmax exp evaluations per core at 1 elem/lane/
cycle is the hard floor, ~240us; PE/DVE/DMA work hides underneath):
  - Matmuls in float32r (FP22 multiply, 1-pass full-rate on the PE); the
    AV stage uses bf16 operands (the fused f32r weight-load path cannot
    target col-offset PSUM, and mixed f32r/bf16 operands are rejected).
  - sim is computed TRANSPOSED: simT[j, i] = sum_d k[d,j] q[d,i], via
    4-way row-packed K=32 matmuls (one per head, tile_position=(32h,0)),
    so no transposes of the big attention matrix are ever needed.
  - softmax: max-subtraction is skipped (|scale*sim| <~ 20 always, exp is
    safe in f32); scale is folded into the ScalarE exp activation.
  - denominator: V^T is augmented with a ones column then zero-padded to
    M=64, so the AV matmul computes sum_j exp*v AND sum_j exp in one pass.
  - AV: out^T[d_aug, i] = sum_j vTaug[j, d_aug] expT[j, i], accumulated
    over j tiles in PSUM; heads col-packed in pairs at tile_position
    (0,0)/(0,64). Every matmul accumulation group owns whole PSUM banks
    (has_written zeroing is 2KB-region granular, NOT per element).
  - normalization + projection emit outT [i, o] so the per-query softmax
    denominator is applied with plain DVE ops; host transposes back.
  - PSUM budget (8 banks): sim quad [128,2048] = 4 banks (single-buffered;
    the sim->exp WAR serialization ~380ns/step is the price), AV pair
    accumulators 2x[128,1024] = 4 banks.
"""

import numpy as np

import concourse.bacc as bacc
import concourse.bass as bass
import concourse.mybir as mybir
import concourse.tile as tile
from concourse.bass_utils import run_bass_kernel_spmd


F32 = mybir.dt.float32
F32R = mybir.dt.float32r
BF16 = mybir.dt.bfloat16

HEADS = 4
DH = 32                      # dim per head
C = 256                      # input channels
NJ = 4096                    # keys per batch (64*64)
NI = 2048                    # queries per core (half of 4096)
JT = 128                     # j tile (partition dim of simT)
NJT = NJ // JT               # 32 j tiles
NT = 512                     # i tile for sim/exp/AV matmuls
CHUNK = 1024                 # i chunk held in AV psum accumulators
NCHUNK = NI // CHUNK         # 2
SCALE = float(DH) ** -0.5
BW = NJ + NI + 3 * 128       # blob256 width


def build_kernel(dbg=False):
    nc = bacc.Bacc("TRN2", debug=False, num_devices=8)

    # blob256 columns: [wqkvT (384) | xq (2048) | x (4096)] - weights and
    # query slice first so the q projection can start while x still streams
    # blob128 columns: [woutA (256) | woutB (256) | bias replicated (256)]
    blob256_d = nc.dram_tensor("blob256", [C, BW], F32R, kind="ExternalInput").ap()
    blob128_d = nc.dram_tensor("blob128", [128, 3 * C], F32R, kind="ExternalInput").ap()
    out_d = nc.dram_tensor("out_t", [NI, C], F32, kind="ExternalOutput").ap()
    # DRAM bounce buffer for partition-broadcasting softmax reciprocals
    # (SBUF->SBUF DMA cannot have a zero partition step on the source).
    rscr_d = nc.dram_tensor("rbscratch", [NCHUNK, 4, CHUNK], F32).ap()
    if dbg:
        dumps = {n: nc.dram_tensor("dump_" + n, s, d, kind="ExternalOutput").ap()
                 for n, s, d in [
                     ("q", [128, NI], F32), ("k", [128, NJ], F32),
                     ("vT", [128, NJT * HEADS * 64], BF16),
                     ("h1", [128, NI], F32), ("h2", [128, NI], F32),
                     ("rb1", [128, CHUNK], F32), ("rb2", [128, CHUNK], F32)]}

    with tile.TileContext(nc) as tc:
        with (
            tc.tile_pool(name="singles", bufs=1) as singles,
            tc.tile_pool(name="expp", bufs=3) as expp,
            tc.tile_pool(name="exp6", bufs=10) as exp6,
            tc.tile_pool(name="outp", bufs=3) as outp,
            tc.tile_pool(name="psim", bufs=1, space="PSUM") as psim,
            tc.tile_pool(name="pav", bufs=2, space="PSUM") as pav,
        ):
            # ---- resident SBUF tensors ----
            blob_sb = singles.tile([128, 2, BW], F32R)    # w|xq|x, 2 c-tiles
            w_sb = blob_sb[:, :, 0:3 * 128]
            xq_sb = blob_sb[:, :, 3 * 128:3 * 128 + NI]
            x_sb = blob_sb[:, :, 3 * 128 + NI:BW]
            b128_sb = singles.tile([128, 3 * C], F32R)
            woutA_sb = b128_sb[:, 0:C]
            woutB_sb = b128_sb[:, C:2 * C]
            bias_sb = b128_sb[:, 2 * C:3 * C]
            q_sb = singles.tile([128, NI], F32R)          # q rows = 4h x 32d
            k_sb = singles.tile([128, NJ], F32R)
            # vT padded to 64 cols: [v dims (32) | ones (1) | zeros (31)]
            # (M=64 keeps the (0,64) col-tiled AV matmul ISA-valid; matmul
            # cost is N-bound so the padding is free)
            # bf16: the fused f32r weight-load path cannot target col-offset
            # PSUM (tile_position (0,64)); bf16 uses the normal LDWEIGHTS path
            vT_sb = singles.tile([128, NJT, HEADS, 64], BF16)
            h1_sb = singles.tile([128, NI], F32R)         # heads 0/1 at rows 0-31/64-95
            h2_sb = singles.tile([128, NI], F32R)         # heads 2/3 at rows 0-31/64-95
            rb1_sb = singles.tile([128, CHUNK], F32)     # 1/denom bcast for h1 rows
            rb2_sb = singles.tile([128, CHUNK], F32)

            # single SWDGE queue -> one semaphore for all initial loads
            # (HWDGE round-robins queues and early matmuls then exceed the
            # per-instruction sync-wait slot limit)
            # One contiguous DMA per c-tile: every matmul then transitively
            # depends on exactly ONE DMA (walrus allows only ~2 semaphore
            # waits per instruction, so the wait sets must stay tiny).
            W0 = 3 * 128 + NI          # w + xq prefix
            for ct in range(2):
                nc.sync.dma_start(out=blob_sb[:, ct, 0:W0],
                                  in_=blob256_d[ct * 128:(ct + 1) * 128, 0:W0])
            for ct in range(2):
                for xh in range(4):
                    lo = W0 + xh * (NJ // 4)
                    nc.sync.dma_start(out=blob_sb[:, ct, lo:lo + NJ // 4],
                                      in_=blob256_d[ct * 128:(ct + 1) * 128,
                                                    lo:lo + NJ // 4])
            nc.sync.dma_start(out=b128_sb, in_=blob128_d)

            nc.vector.memset(h1_sb[:, :].bitcast(F32), 0.0)  # unused rows stay 0
            nc.vector.memset(h2_sb[:, :].bitcast(F32), 0.0)
            nc.vector.memset(vT_sb, 0.0)                # zero padding
            nc.vector.memset(vT_sb[:, :, :, DH], 1.0)   # ones column
            nc.vector.memset(rb1_sb, 0.0)
            nc.vector.memset(rb2_sb, 0.0)

            # trigger the ScalarE exp table load (~2.7us) during phase 1
            # instead of at the first real softmax activation
            warm = singles.tile([1, 1], F32)
            nc.vector.memset(warm, 0.0)
            nc.scalar.activation(warm, warm, mybir.ActivationFunctionType.Exp)

            # ---- phase 1: qkv projections ----
            # ordered to match DMA arrival: q needs only w+xq (first DMAs),
            # k-half0/vT(0-15) need x-half0, the rest needs x-half1
            for qg in range(2):
                psq = psim.tile([128, NI // 2], F32, tag="simA" if qg == 0 else "simB")
                for nt in range(2):
                    col = qg * 1024 + nt * 512
                    for ct in range(2):
                        nc.tensor.matmul(
                            psq[:, nt * 512:(nt + 1) * 512],
                            lhsT=w_sb[:, ct, 0:128],
                            rhs=xq_sb[:, ct, col:col + 512],
                            start=(ct == 0), stop=(ct == 1),
                        )
                nc.vector.tensor_copy(q_sb[:, qg * 1024:(qg + 1) * 1024], psq)

            def emit_k_half(half):
                for kg in range(2):
                    psk = psim.tile([128, NI // 2], F32,
                                    tag="simA" if kg == 0 else "simB")
                    for nt in range(2):
                        col = half * NI + kg * 1024 + nt * 512
                        for ct in range(2):
                            nc.tensor.matmul(
                                psk[:, nt * 512:(nt + 1) * 512],
                                lhsT=w_sb[:, ct, 128:256],
                                rhs=x_sb[:, ct, col:col + 512],
                                start=(ct == 0), stop=(ct == 1),
                            )
                    nc.vector.tensor_copy(
                        k_sb[:, half * NI + kg * 1024:half * NI + (kg + 1) * 1024],
                        psk)

            def emit_vt_range(lo, hi):
                # vT[j, vc] = sum_c x[c, j] wv[vc, c], one [128, 128] tile per jt
                for jt in range(lo, hi):
                    psv = pav.tile([128, 128], F32, tag="av")
                    for ct in range(2):
                        nc.tensor.matmul(
                            psv,
                            lhsT=x_sb[:, ct, jt * JT:(jt + 1) * JT],
                            rhs=w_sb[:, ct, 256:384],
                            start=(ct == 0), stop=(ct == 1),
                        )
                    nc.vector.tensor_copy(vT_sb[:, jt, :, 0:DH], psv)

            emit_k_half(0)
            emit_vt_range(0, NJT // 2)
            emit_k_half(1)
            emit_vt_range(NJT // 2, NJT)

            # ---- phase 2: attention main loop ----
            for chunk in range(NCHUNK):
                co = chunk * CHUNK
                avA = pav.tile([128, CHUNK], F32, tag="av")  # heads 0 @0-32, 1 @64-96
                avB = pav.tile([128, CHUNK], F32, tag="av")  # heads 2 @0-32, 3 @64-96
                def emit_av(ex, jt, nt):
                    for h in range(HEADS):
                        av = avA if h < 2 else avB
                        po = 64 * (h % 2)
                        nc.tensor.matmul(
                            av[po:po + 64, nt * NT:(nt + 1) * NT],
                            lhsT=vT_sb[:, jt, h, :],
                            rhs=ex[:, h * NT:(h + 1) * NT],
                            start=(jt == 0), stop=(jt == NJT - 1),
                            tile_position=(0, po),
                            skip_group_check=True,
                        )

                # AV for step s is emitted after sim for step s+1 so the PE
                # unblocks the ScalarE exp (the critical path) first
                # the quad/exp pair is split in two halves over SEPARATE
                # psum tiles (tags simA/simB): the next step's heads-0/1
                # matmuls overlap the current heads-2/3 exp, so the PE never
                # sits on the ScalarE critical path
                pending = None
                for jt in range(NJT):
                    for nt in range(CHUNK // NT):
                        io = co + nt * NT
                        ex = exp6.tile([128, HEADS * NT], BF16, tag="exp")
                        for grp, tag in ((0, "simA"), (1, "simB")):
                            sim = psim.tile([128, 2 * NT], F32, tag=tag)
                            for hi in range(2):
                                h = grp * 2 + hi
                                nc.tensor.matmul(
                                    sim[:, hi * NT:(hi + 1) * NT],
                                    lhsT=k_sb[h * DH:(h + 1) * DH,
                                              jt * JT:(jt + 1) * JT],
                                    rhs=q_sb[h * DH:(h + 1) * DH, io:io + NT],
                                    start=True, stop=True,
                                    tile_position=(h * DH, 0),
                                )
                            nc.scalar.activation(
                                ex[:, grp * 2 * NT:(grp + 1) * 2 * NT], sim,
                                mybir.ActivationFunctionType.Exp, scale=SCALE)
                        if pending is not None:
                            emit_av(*pending)
                        pending = (ex, jt, nt)
                emit_av(*pending)

                # softmax denominators (rows 32 & 96 of the av tiles):
                # bounce the 4 rows through DRAM to pack them into [4, CHUNK]
                # (reciprocal is free-dim bound: one packed call is 4x
                # cheaper than four [1, CHUNK] calls), then reciprocal,
                # bounce back, and partition-broadcast over each head's rows.
                den4 = expp.tile([4, CHUNK], F32, tag="rc")
                rc4 = expp.tile([4, CHUNK], F32, tag="rc")
                dstg = expp.tile([128, 2 * CHUNK], F32, tag="dstg")
                for idx, av in enumerate((avA, avB)):
                    for pi, po in enumerate((0, 64)):
                        h4 = idx * 2 + pi
                        cs = idx * CHUNK
                        # DMA cannot read PSUM: stage the row via DVE
                        # (same partition, pair-tiles split by free offset)
                        nc.vector.tensor_copy(dstg[po + DH:po + DH + 1, cs:cs + CHUNK],
                                              av[po + DH:po + DH + 1, :])
                        nc.sync.dma_start(out=rscr_d[chunk, h4, :],
                                          in_=dstg[po + DH:po + DH + 1, cs:cs + CHUNK])
                nc.sync.dma_start(out=den4, in_=rscr_d[chunk, :, :])
                nc.vector.reciprocal(out=rc4, in_=den4)
                nc.sync.dma_start(out=rscr_d[chunk, :, :], in_=rc4)
                for idx, rb in enumerate((rb1_sb, rb2_sb)):
                    for pi, po in enumerate((0, 64)):
                        h4 = idx * 2 + pi
                        nc.sync.dma_start(
                            out=rb[po:po + DH, :],
                            in_=rscr_d[chunk, h4:h4 + 1, :].to_broadcast((DH, CHUNK)),
                        )
                # fused normalize + PSUM->SBUF copy
                for (av, rb, hsb) in ((avA, rb1_sb, h1_sb), (avB, rb2_sb, h2_sb)):
                    for po in (0, 64):
                        nc.vector.tensor_tensor(
                            out=hsb[po:po + DH, co:co + CHUNK],
                            in0=av[po:po + DH, :],
                            in1=rb[po:po + DH, :],
                            op=mybir.AluOpType.mult,
                        )

                # ---- output projection for this chunk ----
                for it in range(CHUNK // 128):
                    io = co + it * 128
                    pj = pav.tile([128, C], F32, tag="av")
                    nc.tensor.matmul(pj, lhsT=h1_sb[:, io:io + 128],
                                     rhs=woutA_sb, start=True, stop=False)
                    nc.tensor.matmul(pj, lhsT=h2_sb[:, io:io + 128],
                                     rhs=woutB_sb, start=False, stop=True)
                    ot = outp.tile([128, C], F32, tag="out")
                    nc.vector.tensor_tensor(out=ot, in0=pj, in1=bias_sb,
                                            op=mybir.AluOpType.add)
                    nc.sync.dma_start(out=out_d[io:io + 128, :], in_=ot)

            if dbg:
                nc.sync.dma_start(out=dumps["q"], in_=q_sb[:, :].bitcast(F32))
                nc.sync.dma_start(out=dumps["k"], in_=k_sb[:, :].bitcast(F32))
                nc.sync.dma_start(out=dumps["vT"],
                                  in_=vT_sb[:, :, :, :].rearrange("p a b c -> p (a b c)"))
                nc.sync.dma_start(out=dumps["h1"], in_=h1_sb[:, :].bitcast(F32))
                nc.sync.dma_start(out=dumps["h2"], in_=h2_sb[:, :].bitcast(F32))
                nc.sync.dma_start(out=dumps["rb1"], in_=rb1_sb)
                nc.sync.dma_start(out=dumps["rb2"], in_=rb2_sb)

    nc.compile()
    return nc


_NC = None


def _get_nc():
    global _NC
    if _NC is None:
        _NC = build_kernel()
    return _NC


def make_in_maps(x, w_qkv, w_out, b_out):
    x = np.ascontiguousarray(np.asarray(x, dtype=np.float32))
    w_qkv = np.asarray(w_qkv, dtype=np.float32)
    w_out = np.asarray(w_out, dtype=np.float32)
    b_out = np.asarray(b_out, dtype=np.float32)

    wqkvT = w_qkv.T                                       # [256, 384]
    woutT = w_out.T                                       # [128 hidden, 256]
    # projection weights permuted to the AV psum partition layout:
    # A: rows 0-31 = head0, rows 64-95 = head1; B: head2, head3; rest zero
    woutA = np.zeros((128, C), np.float32)
    woutB = np.zeros((128, C), np.float32)
    woutA[0:32] = woutT[0:32]
    woutA[64:96] = woutT[32:64]
    woutB[0:32] = woutT[64:96]
    woutB[64:96] = woutT[96:128]
    blob128 = np.ascontiguousarray(
        np.concatenate([woutA, woutB,
                        np.broadcast_to(b_out[None, :], (128, C))], axis=1))

    in_maps = []
    for core in range(8):
        b, qh = divmod(core, 2)
        xb = x[b].reshape(C, NJ)
        xqb = xb[:, qh * NI:(qh + 1) * NI]
        blob256 = np.ascontiguousarray(
            np.concatenate([wqkvT, xqb, xb], axis=1))
        in_maps.append({"blob256": blob256, "blob128": blob128})
    return in_maps


def run_spmd(x, w_qkv, w_out, b_out, **kw):
    nc = _get_nc()
    in_maps = make_in_maps(x, w_qkv, w_out, b_out)
    return run_bass_kernel_spmd(nc, in_maps, core_ids=list(range(8)), **kw)


def assemble(results):
    out = np.empty((4, C, NJ), np.float32)
    for core in range(8):
        b, qh = divmod(core, 2)
        out[b, :, qh * NI:(qh + 1) * NI] = results[core]["out_t"].T
    return out.reshape(4, C, 64, 64)


def kernel(x, w_qkv, w_out, b_out):
    res = run_spmd(x, w_qkv, w_out, b_out)
    return assemble(res.results)
